# revision 30
# baseline (speedup 1.0000x reference)
"""Trainium2 Bass kernel for nn_BAZ_Network (dense CNN + cov/eig head).

Data-parallel over 8 NeuronCores: 128 samples each.

Launch 1 (per core), software-pipelined over 64 sample-pairs:
  conv trunk as G-packed banded-weight matmuls (bf16, fp32 PSUM), with
  conv biases folded into the matmuls via a ones-row in the rhs (conv0,
  conv1).  Postprocess per (E,O) parity pair is two fused ops over a
  2-sample two-PSUM-bank 3D access pattern:
    op1 (Act):  tE = relu(psE + b)           PSUM -> SBUF bf16
    op2 (DVE):  s  = max(psO + b, tE)        = relu(max(E,O)+b), the
                 maxpool, relu, bias and bf16 cast in one instruction.
  conv3 is dense-P5: one psum accumulates 5 chunked matmuls over the
  s3 pair-column layout (64/128/128/128/64 contraction rows); M=128
  packs pool-pair firsts in rows 0:60 and seconds in 64:124 so the
  maxpool merges across the aligned partition halves (junk rows are
  zero-weighted and killed by zero rows in WFC).  The FC contraction
  of the conv features against wl0[:, :7500] runs per-block.
  Stage stagger: conv0(p) | conv1(p-1) | conv2(p-2) | conv3 at even p |
  FC at p=35/67, which hides the halo-DMA and PSUM-evacuation latency.
Host: fp32 covariance (same einsum as the reference; cheaper than
  streaming x to the device a second time) + branch-exact fp32
  netlib-LAPACK ssyevd clone for the 3x3 eigh (required to reproduce
  jnp.linalg.eigh eigenvector signs).
Launch 2 (per core): eig-feature head: 1x1 conv (wc) + relu, remaining
  FC columns wl0[:, 7500:], bias+relu, final linear wl1.
"""

import os
import sys
import time
import numpy as np
import ml_dtypes

sys.path.insert(0, "/opt/trn_rl_repo")
os.environ["BASS_NEVER_TRACE"] = "1"

import concourse.bass as bass  # noqa: E402
import concourse.tile as tile  # noqa: E402
import concourse.mybir as mybir  # noqa: E402
from concourse import bacc  # noqa: E402
from concourse.bass_utils import run_bass_kernel_spmd  # noqa: E402

F32 = mybir.dt.float32
BF16 = mybir.dt.bfloat16
AOP = mybir.AluOpType
ACTF = mybir.ActivationFunctionType
BF = ml_dtypes.bfloat16

NCORES = 8
NS = 128          # samples per core
BN = 8            # samples per block
NBLK = NS // BN
NPAIR = NS // 2   # 64 sample-pairs, the pipeline unit
L0 = 6000

FLAGS = {"cov": True, "halo": True, "conv3": True, "fc": True,
         "conv0": True, "conv1": True, "conv2": True}
LAST_EXEC_NS = [None, None]
LAST_WALL_S = [None, None]
_CACHE = {}


# ---------------------------------------------------------------- eigh ----
# fp32 netlib-LAPACK ssyevd clone for n=3 (jobz='V', uplo='L').
# Matches jaxlib's CPU eigh (LAPACK >= 3.10 slartg) bit-closely: 0/3072
# eigenvector sign mismatches on the problem distribution.

_F = np.float32
_EPS = _F(np.finfo(np.float32).eps) * _F(0.5)
_EPS2 = _EPS * _EPS
_SAFMIN = _F(np.finfo(np.float32).tiny)


def _slapy2(x, y):
    xa, ya = abs(x), abs(y)
    w, z = max(xa, ya), min(xa, ya)
    if z == 0:
        return w
    return _F(w * _F(np.sqrt(_F(_F(1.0) + _F(_F(z / w) * _F(z / w))))))


def _sign(a, b):
    return abs(a) if b >= 0 else -abs(a)


def _slartg(f, g):
    if g == _F(0.0):
        return _F(1.0), _F(0.0), f
    if f == _F(0.0):
        return _F(0.0), _sign(_F(1.0), g), abs(g)
    d = _F(np.sqrt(_F(f * f + g * g)))
    c = _F(abs(f) / d)
    r = _sign(d, f)
    s = _F(g / r)
    return c, s, r


def _slaev2(a, b, c):
    sm = _F(a + c)
    df = _F(a - c)
    adf = abs(df)
    tb = _F(b + b)
    ab = abs(tb)
    acmx, acmn = (a, c) if abs(a) > abs(c) else (c, a)
    if adf > ab:
        t = _F(ab / adf)
        rt = _F(adf * _F(np.sqrt(_F(_F(1.0) + _F(t * t)))))
    elif adf < ab:
        t = _F(adf / ab)
        rt = _F(ab * _F(np.sqrt(_F(_F(1.0) + _F(t * t)))))
    else:
        rt = _F(ab * _F(np.sqrt(_F(2.0))))
    if sm < 0:
        rt1 = _F(_F(0.5) * _F(sm - rt))
        sgn1 = -1
        rt2 = _F(_F(_F(acmx / rt1) * acmn) - _F(_F(b / rt1) * b))
    elif sm > 0:
        rt1 = _F(_F(0.5) * _F(sm + rt))
        sgn1 = 1
        rt2 = _F(_F(_F(acmx / rt1) * acmn) - _F(_F(b / rt1) * b))
    else:
        rt1 = _F(_F(0.5) * rt)
        rt2 = _F(_F(-0.5) * rt)
        sgn1 = 1
    if df >= 0:
        cs = _F(df + rt)
        sgn2 = 1
    else:
        cs = _F(df - rt)
        sgn2 = -1
    acs = abs(cs)
    if acs > ab:
        ct = _F(-tb / cs)
        sn1 = _F(_F(1.0) / _F(np.sqrt(_F(_F(1.0) + _F(ct * ct)))))
        cs1 = _F(ct * sn1)
    else:
        if ab == 0:
            cs1, sn1 = _F(1.0), _F(0.0)
        else:
            tn = _F(-cs / tb)
            cs1 = _F(_F(1.0) / _F(np.sqrt(_F(_F(1.0) + _F(tn * tn)))))
            sn1 = _F(tn * cs1)
    if sgn1 == sgn2:
        cs1, sn1 = -sn1, cs1
    return rt1, rt2, cs1, sn1


def _ssytrd3(A):
    a00, a10, a20 = A[0, 0], A[1, 0], A[2, 0]
    a11, a21, a22 = A[1, 1], A[2, 1], A[2, 2]
    xnorm = abs(a20)
    if xnorm == _F(0.0):
        beta, v2, tau = a10, a20, _F(0.0)
    else:
        beta = -_sign(_slapy2(a10, xnorm), a10)
        tau = _F(_F(beta - a10) / beta)
        v2 = _F(a20 * _F(_F(1.0) / _F(a10 - beta)))
    e0 = beta
    if tau != _F(0.0):
        x0 = _F(_F(tau * a11) + _F(tau * _F(a21 * v2)))
        x1 = _F(_F(tau * a21) + _F(_F(tau * v2) * a22))
        sdot = _F(_F(x0 * _F(1.0)) + _F(x1 * v2))
        alpha = _F(_F(_F(-0.5) * tau) * sdot)
        w0 = _F(x0 + _F(alpha * _F(1.0)))
        w1 = _F(x1 + _F(alpha * v2))
        t1, t2 = -w0, _F(-1.0)
        a11 = _F(_F(a11 + _F(_F(1.0) * t1)) + _F(w0 * t2))
        a21 = _F(_F(a21 + _F(v2 * t1)) + _F(w1 * t2))
        t1b, t2b = -w1, -v2
        a22 = _F(_F(a22 + _F(v2 * t1b)) + _F(w1 * t2b))
    d = np.array([a00, a11, a22], np.float32)
    e = np.array([e0, a21, 0.0], np.float32)
    return d, e, v2, tau


def _ssteqr3(d, e):
    n = 3
    Z = np.eye(3, dtype=np.float32)
    wc = np.zeros(2, np.float32)
    ws = np.zeros(2, np.float32)
    nmaxit, jtot = 90, 0

    def lasr_b(l, m):
        for j in range(m - 1, l - 1, -1):
            c, s = wc[j - 1], ws[j - 1]
            if c != _F(1.0) or s != _F(0.0):
                for i in range(3):
                    t = Z[i, j]
                    Z[i, j] = _F(_F(c * t) - _F(s * Z[i, j - 1]))
                    Z[i, j - 1] = _F(_F(s * t) + _F(c * Z[i, j - 1]))

    def lasr_f(m, l):
        for j in range(m, l):
            c, s = wc[j - 1], ws[j - 1]
            if c != _F(1.0) or s != _F(0.0):
                for i in range(3):
                    t = Z[i, j]
                    Z[i, j] = _F(_F(c * t) - _F(s * Z[i, j - 1]))
                    Z[i, j - 1] = _F(_F(s * t) + _F(c * Z[i, j - 1]))

    l1 = 1
    while True:
        if l1 > n:
            break
        if l1 > 1:
            e[l1 - 2] = _F(0.0)
        m = n
        for mm in range(l1, n):
            tst = abs(e[mm - 1])
            if tst == _F(0.0):
                m = mm
                break
            if tst <= _F(_F(_F(np.sqrt(abs(d[mm - 1]))) *
                            _F(np.sqrt(abs(d[mm])))) * _EPS):
                e[mm - 1] = _F(0.0)
                m = mm
                break
        l = l1
        lend = m
        l1 = m + 1
        if lend == l:
            continue
        if abs(d[lend - 1]) < abs(d[l - 1]):
            lend, l = l, lend
        if lend > l:
            while True:  # QL
                m = lend
                if l != lend:
                    for mm in range(l, lend):
                        tst = _F(abs(e[mm - 1]) * abs(e[mm - 1]))
                        if tst <= _F(_F(_F(_EPS2 * abs(d[mm - 1])) *
                                        abs(d[mm])) + _SAFMIN):
                            m = mm
                            break
                if m < lend:
                    e[m - 1] = _F(0.0)
                p = d[l - 1]
                if m == l:
                    d[l - 1] = p
                    l += 1
                    if l <= lend:
                        continue
                    break
                if m == l + 1:
                    rt1, rt2, c, s = _slaev2(d[l - 1], e[l - 1], d[l])
                    wc[l - 1] = c
                    ws[l - 1] = s
                    lasr_b(l, l + 1)
                    d[l - 1] = rt1
                    d[l] = rt2
                    e[l - 1] = _F(0.0)
                    l += 2
                    if l <= lend:
                        continue
                    break
                if jtot == nmaxit:
                    break
                jtot += 1
                g = _F(_F(d[l] - p) / _F(_F(2.0) * e[l - 1]))
                r = _slapy2(g, _F(1.0))
                g = _F(_F(d[m - 1] - p) + _F(e[l - 1] / _F(g + _sign(r, g))))
                s = _F(1.0)
                c = _F(1.0)
                p = _F(0.0)
                for i in range(m - 1, l - 1, -1):
                    f = _F(s * e[i - 1])
                    b = _F(c * e[i - 1])
                    c, s, r = _slartg(g, f)
                    if i != m - 1:
                        e[i] = r
                    g = _F(d[i] - p)
                    r = _F(_F(_F(d[i - 1] - g) * s) + _F(_F(_F(2.0) * c) * b))
                    p = _F(s * r)
                    d[i] = _F(g + p)
                    g = _F(_F(c * r) - b)
                    wc[i - 1] = c
                    ws[i - 1] = -s
                lasr_b(l, m)
                d[l - 1] = _F(d[l - 1] - p)
                e[l - 1] = g
        else:
            while True:  # QR
                m = lend
                if l != lend:
                    for mm in range(l, lend, -1):
                        tst = _F(abs(e[mm - 2]) * abs(e[mm - 2]))
                        if tst <= _F(_F(_F(_EPS2 * abs(d[mm - 1])) *
                                        abs(d[mm - 2])) + _SAFMIN):
                            m = mm
                            break
                if m > lend:
                    e[m - 2] = _F(0.0)
                p = d[l - 1]
                if m == l:
                    d[l - 1] = p
                    l -= 1
                    if l >= lend:
                        continue
                    break
                if m == l - 1:
                    rt1, rt2, c, s = _slaev2(d[l - 2], e[l - 2], d[l - 1])
                    wc[m - 1] = c
                    ws[m - 1] = s
                    lasr_f(m, l)
                    d[l - 2] = rt1
                    d[l - 1] = rt2
                    e[l - 2] = _F(0.0)
                    l -= 2
                    if l >= lend:
                        continue
                    break
                if jtot == nmaxit:
                    break
                jtot += 1
                g = _F(_F(d[l - 2] - p) / _F(_F(2.0) * e[l - 2]))
                r = _slapy2(g, _F(1.0))
                g = _F(_F(d[m - 1] - p) + _F(e[l - 2] / _F(g + _sign(r, g))))
                s = _F(1.0)
                c = _F(1.0)
                p = _F(0.0)
                for i in range(m, l):
                    f = _F(s * e[i - 1])
                    b = _F(c * e[i - 1])
                    c, s, r = _slartg(g, f)
                    if i != m:
                        e[i - 2] = r
                    g = _F(d[i - 1] - p)
                    r = _F(_F(_F(d[i] - g) * s) + _F(_F(_F(2.0) * c) * b))
                    p = _F(s * r)
                    d[i - 1] = _F(g + p)
                    g = _F(_F(c * r) - b)
                    wc[i - 1] = c
                    ws[i - 1] = s
                lasr_f(m, l)
                d[l - 1] = _F(d[l - 1] - p)
                e[l - 2] = g
        if jtot >= nmaxit:
            break
    for ii in range(2, n + 1):
        i = ii - 1
        k = i
        p = d[i - 1]
        for j in range(ii, n + 1):
            if d[j - 1] < p:
                k = j
                p = d[j - 1]
        if k != i:
            d[k - 1] = d[i - 1]
            d[i - 1] = p
            tmp = Z[:, k - 1].copy()
            Z[:, k - 1] = Z[:, i - 1]
            Z[:, i - 1] = tmp
    return d, Z


def _eigh3_batch(covs):
    n = covs.shape[0]
    W = np.empty((n, 3), np.float32)
    V = np.empty((n, 3, 3), np.float32)
    for i in range(n):
        d, e, v2, tau = _ssytrd3(covs[i])
        w, Z = _ssteqr3(d, e)
        if tau != _F(0.0):
            for j in range(3):
                vtz = _F(Z[1, j] + _F(v2 * Z[2, j]))
                tvz = _F(tau * vtz)
                Z[1, j] = _F(Z[1, j] - tvz)
                Z[2, j] = _F(Z[2, j] - _F(v2 * tvz))
        W[i] = w
        V[i] = Z
    return W, V


# ------------------------------------------------------------- weights ----

def _prep_weights(ins):
    """Host-side packing of the model weights into device layouts.

    Strided column-pair scheme (see the layer maps below); conv0/conv1
    biases ride a ones-row in the rhs so the matmul itself adds them.
    """
    w0, w1, w2, w3 = ins["w0"], ins["w1"], ins["w2"], ins["w3"]
    b0, b1 = np.asarray(ins["b0"], np.float32), np.asarray(ins["b1"],
                                                           np.float32)

    d = {}
    # conv0: window rows (c:3, j:9): even cols l = 8q-1+j, odd l = 8q+j;
    # k = j - 2g for output slot g; row 27 = ones -> bias.
    W0E = np.zeros((31, 80), np.float32)
    W0O = np.zeros((31, 80), np.float32)
    for c in range(3):
        for jj in range(10):
            for g in range(4):
                for o in range(20):
                    k = jj - 2 * g
                    if 0 <= k < 3:
                        W0E[c * 10 + jj, g * 20 + o] = w0[o, c, k]
                    k = jj - 1 - 2 * g
                    if 0 <= k < 3:
                        W0O[c * 10 + jj, g * 20 + o] = w0[o, c, k]
    W0E[30, :] = np.tile(b0, 4)
    W0O[30, :] = np.tile(b0, 4)
    d["W0E"] = W0E.astype(BF)
    d["W0O"] = W0O.astype(BF)

    def s1_rows(with_hl, with_hr):
        rows = [(g * 20, 20, g) for g in range(4)]
        if with_hl:
            rows.append((80, 20, -1))
        if with_hr:
            rows.append((100, 20, 4))
        return rows

    def mk(w, blocks, Ghalf, parity, Cout, shift, colbase=None):
        Cin = w.shape[1]
        K = max(rb + Cin for rb, _, _ in blocks)
        if colbase is None:
            colbase = [g * Cout for g in range(Ghalf)]
        W = np.zeros((K, max(colbase) + Cout), np.float32)
        for rb, _, lrel in blocks:
            for g in range(Ghalf):
                pos = 2 * g + parity
                k = (lrel + shift) - pos + 1
                if 0 <= k < 3:
                    for ci in range(Cin):
                        W[rb + ci, colbase[g] + np.arange(Cout)] = w[:, ci, k]
        return W

    # conv1 output M-order: g0->0, g1->64, g2->96, g3->32 (C1B) so conv2's
    # boundary reads sit at legal rhs bases.
    C1B = [0, 64, 96, 32]

    # baseline-layout W1 blocks, then re-rowed for the s1 layout with the
    # ones row at 80: main [0:80], ones 80, hl [81:101], hr [101:121].
    w1e1_base = mk(w1, s1_rows(True, False), 4, 0, 32, 0, C1B)   # [100,128]
    w1e2 = mk(w1, [(rb, 20, lr + 4) for rb, _, lr in
                   s1_rows(False, False)], 4, 0, 32, 0, C1B)     # [80,128]
    w1o1 = mk(w1, s1_rows(False, False), 4, 1, 32, 0, C1B)       # [80,128]
    w1o2_blocks = ([(g * 20, 20, g + 4) for g in range(4)] +
                   [(80, 20, 1000), (100, 20, 8)])
    w1o2_base = mk(w1, w1o2_blocks, 4, 1, 32, 0, C1B)            # [120,128]
    b1t = np.tile(b1, 4)
    W1e1 = np.zeros((101, 128), np.float32)
    W1e1[0:80] = w1e1_base[0:80]
    W1e1[80] = b1t
    W1e1[81:101] = w1e1_base[80:100]
    W1o2 = np.zeros((121, 128), np.float32)
    W1o2[0:80] = w1o2_base[0:80]
    W1o2[80] = b1t
    W1o2[101:121] = w1o2_base[100:120]
    d["W1e1"] = W1e1.astype(BF)
    d["W1e2"] = w1e2.astype(BF)
    d["W1o1"] = w1o1.astype(BF)
    d["W1o2"] = W1o2.astype(BF)

    # conv2 (G=4, Ghalf=2, Cout=64): stored2 rows (g:4, o:32)->128
    s2_main = [(0, 32, 0), (64, 32, 1), (96, 32, 2), (32, 32, 3)]
    d["W2e1"] = mk(w2, [(0, 32, -1)], 2, 0, 64, 0).astype(BF)
    d["W2e2"] = mk(w2, s2_main, 2, 0, 64, 0).astype(BF)
    d["W2o1"] = mk(w2, s2_main, 2, 1, 64, 0).astype(BF)
    d["W2o2"] = mk(w2, [(0, 32, 4)], 2, 1, 64, 0).astype(BF)

    # conv3 dense-P5: out block u = dense positions {6u..6u+5}, window
    # {6u-1..6u+6} read as 5 accumulated chunks from s3 pair-columns
    # (col v = positions {2v-2, 2v-1}; g=0 rows 0:64, g=1 rows 64:128):
    #   p0: col 3u   g1 (pos 6u-1),  p1: col 3u+1 (6u, 6u+1),
    #   p2: col 3u+2 (6u+2, 6u+3),   p3: col 3u+3 (6u+4, 6u+5),
    #   p4: col 3u+4 g0 (pos 6u+6).
    # M = 128: rows 0-59 firsts-of-pool-pairs (j = 2*j2), 64-123 seconds
    # (j = 2*j2+1), 60-63/124-127 zero-weight junk (killed by WFC zeros).
    w3p = np.zeros((128, 5 * 128), np.float32)
    PASS_POS = [{1: -1}, {0: 0, 1: 1}, {0: 2, 1: 3}, {0: 4, 1: 5}, {0: 6}]
    for pi, gmap in enumerate(PASS_POS):
        for g, pos_rel in gmap.items():
            for m in range(128):
                sub, up = m % 64, m // 64
                if sub >= 60:
                    continue
                j = 2 * (sub // 20) + up
                k = pos_rel - j + 1
                if 0 <= k < 3:
                    o = sub % 20
                    for ci in range(64):
                        w3p[g * 64 + ci, pi * 128 + m] = w3[o, ci, k]
    d["W3P"] = w3p.astype(BF)

    # fc: s4 rows (up:2 pad64, j2:3, o:20), col w: feature (o, l4=6w+3up+j2)
    wl0 = ins["wl0"]
    WFC = np.zeros((128, 63 * 100), np.float32)
    for w in range(63):
        for up in range(2):
            for j2 in range(3):
                l4 = 6 * w + 3 * up + j2
                if l4 < 375:
                    for o in range(20):
                        WFC[up * 64 + j2 * 20 + o, w * 100:(w + 1) * 100] = \
                            wl0[:, o * 375 + l4]
    d["WFC"] = WFC.astype(BF)

    d["B2"] = np.tile(ins["b2"], 2).astype(np.float32)[:, None]   # [128]
    b3h = np.concatenate([np.tile(ins["b3"], 3), np.zeros(4)])
    d["B3"] = np.concatenate([b3h, b3h]).astype(np.float32)[:, None]  # [128]
    # launch 2
    d["wcT"] = ins["wc"][:, :, 0].T.astype(np.float32).copy()      # [3, 20]
    d["bc"] = ins["bc"].astype(np.float32)[:, None]                # [20, 1]
    w0b = np.zeros((7, 20, 100), np.float32)
    for t in range(7):
        for o in range(20):
            w0b[t, o] = ins["wl0"][:, 7500 + o * 7 + t]
    d["w0bT"] = w0b
    d["bl0"] = ins["bl0"].astype(np.float32)[:, None]              # [100, 1]
    d["wl1T"] = ins["wl1"].T.astype(np.float32).copy()             # [100, 2]
    d["bl1"] = ins["bl1"].astype(np.float32)[:, None]              # [2, 1]
    return d


# ------------------------------------------------------------- launch 1 ----

def _build_launch1():
    nc = bacc.Bacc("TRN2", target_bir_lowering=False, debug=False,
                   num_devices=NCORES)
    dram = {}
    for nm, shape, dt in [
        ("x_win", [31, NS, 750], BF16),
        ("W0E", [31, 80], BF16), ("W0O", [31, 80], BF16),
        ("W1e1", [101, 128], BF16), ("W1e2", [80, 128], BF16),
        ("W1o1", [80, 128], BF16), ("W1o2", [121, 128], BF16),
        ("W2e1", [32, 128], BF16), ("W2e2", [128, 128], BF16),
        ("W2o1", [128, 128], BF16), ("W2o2", [32, 128], BF16),
        ("W3P", [128, 640], BF16),
        ("WFC", [128, 6300], BF16),
        ("B2", [128, 1], F32), ("B3", [128, 1], F32),
        ("INIT1", [1, BN, 750], BF16), ("INITZ", [128, BN, 1], BF16),
        ("INITZ4", [128, BN, 4], BF16), ("INITS4", [64, NS, 1], BF16),
    ]:
        dram[nm] = nc.dram_tensor(nm, shape, dt, kind="ExternalInput").ap()
    out_p0 = nc.dram_tensor("partial0", [100, NS], F32,
                            kind="ExternalOutput").ap()

    with tile.TileContext(nc) as tc:
        with tc.tile_pool(name="wpool", bufs=1) as wp, \
             tc.tile_pool(name="xw", bufs=2) as xwp, \
             tc.tile_pool(name="s1p", bufs=1) as s1p, \
             tc.tile_pool(name="s2p", bufs=1) as s2p, \
             tc.tile_pool(name="s3p", bufs=1) as s3p, \
             tc.tile_pool(name="s4p", bufs=1) as s4p, \
             tc.tile_pool(name="tep", bufs=4) as tep, \
             tc.tile_pool(name="ps", bufs=4, space="PSUM") as psp:

            xw_tiles = {}

            def issue_xw(b):
                if b >= NBLK or b in xw_tiles:
                    return
                n0 = b * BN
                tw = xwp.tile([31, BN, 750], BF16, tag="xw")
                nc.sync.dma_start(tw[:], dram["x_win"][:, n0:n0 + BN, :])
                xw_tiles[b] = tw

            # The SP DMA queue is FIFO and a DMA holds HWDGE ~0.6us each:
            # preload ONLY what conv0(0)/conv1(0) need, trickle the rest
            # into the pipeline steps below via dma_sched.
            p0sb = wp.tile([100, NS], F32, tag="p0sb")
            s1 = s1p.tile([121, BN, 750], BF16, tag="s1")
            s2 = s2p.tile([128, BN, 377], BF16, tag="s2")
            s3 = s3p.tile([128, BN, 380], BF16, tag="s3")
            s4 = s4p.tile([128, NS, 63], BF16, tag="s4")

            Ws = {}
            for nm in ["W1e1", "W1e2", "W1o1", "W1o2", "W0E", "W0O",
                       "W2e2", "W2o1", "W2o2", "W3P", "WFC"]:
                Ws[nm] = wp.tile(list(dram[nm].shape), BF16, name=nm, tag=nm)
            Wpad = wp.tile([64, 128], BF16, name="W2e1", tag="W2e1")
            Ws["W2e1"] = Wpad[32:64]
            B2t = wp.tile([128, 1], F32, tag="B2")
            B3t = wp.tile([128, 1], F32, tag="B3")

            def dma_w(nm):
                nc.sync.dma_start(Ws[nm][:], dram[nm][:])

            # prologue: conv0(0..1) + conv1(0) prerequisites only
            dma_w("W0E")
            dma_w("W0O")
            issue_xw(0)
            issue_xw(1)
            # ones row + hl/hr edges (engines cannot address partition
            # bases off the 0/32/64/96 grid; DMA can); conv1 prereqs ride
            # the otherwise-idle Act HWDGE queue in parallel
            nc.scalar.dma_start(s1[80:81, :, :], dram["INIT1"][:])
            nc.scalar.dma_start(s1[81:101, :, 0:1], dram["INITZ"][0:20])
            nc.scalar.dma_start(s1[101:121, :, 749:750], dram["INITZ"][0:20])
            for nm in ["W1e1", "W1e2", "W1o1", "W1o2"]:
                nc.scalar.dma_start(Ws[nm][:], dram[nm][:])

            def dmas_step0():
                nc.sync.dma_start(Wpad[32:64], dram["W2e1"][:])
                for nm in ["W2e2", "W2o1", "W2o2"]:
                    dma_w(nm)
                nc.sync.dma_start(B2t[:], dram["B2"][:])
                nc.sync.dma_start(s2[:, :, 0:1], dram["INITZ"][:])
                nc.sync.dma_start(s2[:, :, 376:377], dram["INITZ"][:])

            def dmas_step1():
                dma_w("W3P")
                nc.sync.dma_start(s3[:, :, 0:1], dram["INITZ"][:])

            def dmas_step2():
                nc.sync.dma_start(B3t[:], dram["B3"][:])
                nc.sync.dma_start(s3[:, :, 376:380], dram["INITZ4"][:])
                nc.sync.dma_start(s4[64:128, :, 62:63], dram["INITS4"][:])

            def make_wfc_piece(i):
                c0 = i * 1575
                return lambda: nc.scalar.dma_start(
                    Ws["WFC"][:, c0:c0 + 1575], dram["WFC"][:, c0:c0 + 1575])

            dma_sched = {}
            dma_sched.setdefault(0, []).append(dmas_step0)
            dma_sched.setdefault(1, []).append(dmas_step1)
            dma_sched.setdefault(2, []).append(dmas_step2)
            for i in range(4):
                dma_sched.setdefault(26 + 2 * i, []).append(make_wfc_piece(i))

            def conv0(p):
                blk, nb = p // 4, (p % 4) * 2
                xwt = xw_tiles[blk]
                for ch in range(2):
                    c0 = ch * 375
                    psE = psp.tile([128, 2, 512], F32, tag="ps")
                    psO = psp.tile([128, 2, 512], F32, tag="ps")
                    for i in range(2):
                        nc.tensor.matmul(psE[0:80, i, 0:375], Ws["W0E"][:],
                                         xwt[:, nb + i, c0:c0 + 375],
                                         start=True, stop=True)
                    for i in range(2):
                        nc.tensor.matmul(psO[0:80, i, 0:375], Ws["W0O"][:],
                                         xwt[:, nb + i, c0:c0 + 375],
                                         start=True, stop=True)
                    tE = tep.tile([128, 2, 384], BF16, tag="tE")
                    nc.scalar.activation(tE[0:80, :, 0:375],
                                         psE[0:80, :, 0:375], ACTF.Relu)
                    nc.vector.scalar_tensor_tensor(
                        s1[0:80, nb:nb + 2, c0:c0 + 375],
                        psO[0:80, :, 0:375], 0.0, tE[0:80, :, 0:375],
                        AOP.max, AOP.max)
                # per-pair halo rows for conv1
                if FLAGS["halo"]:
                    nc.sync.dma_start(s1[81:101, nb:nb + 2, 1:750],
                                      s1[60:80, nb:nb + 2, 0:749])
                    nc.sync.dma_start(s1[101:121, nb:nb + 2, 0:749],
                                      s1[0:20, nb:nb + 2, 1:750])

            def conv1(p):
                nb = (p % 4) * 2
                psE = psp.tile([128, 2, 512], F32, tag="ps")
                psO = psp.tile([128, 2, 512], F32, tag="ps")
                for i in range(2):
                    n = nb + i
                    nc.tensor.matmul(psE[0:128, i, 0:375], Ws["W1e1"][:],
                                     s1[0:101, n, 0:750:2],
                                     start=True, stop=False)
                    nc.tensor.matmul(psE[0:128, i, 0:375], Ws["W1e2"][:],
                                     s1[0:80, n, 1:750:2],
                                     start=False, stop=True)
                for i in range(2):
                    n = nb + i
                    nc.tensor.matmul(psO[0:128, i, 0:375], Ws["W1o1"][:],
                                     s1[0:80, n, 0:750:2],
                                     start=True, stop=False)
                    nc.tensor.matmul(psO[0:128, i, 0:375], Ws["W1o2"][:],
                                     s1[0:121, n, 1:750:2],
                                     start=False, stop=True)
                tE = tep.tile([128, 2, 384], BF16, tag="tE")
                nc.scalar.activation(tE[0:128, :, 0:375],
                                     psE[0:128, :, 0:375], ACTF.Relu)
                nc.vector.scalar_tensor_tensor(
                    s2[0:128, nb:nb + 2, 1:376],
                    psO[0:128, :, 0:375], 0.0, tE[0:128, :, 0:375],
                    AOP.max, AOP.max)

            def conv2(p):
                nb = (p % 4) * 2
                psE = psp.tile([128, 2, 512], F32, tag="ps")
                psO = psp.tile([128, 2, 512], F32, tag="ps")
                for i in range(2):
                    n = nb + i
                    nc.tensor.matmul(psE[0:128, i, 0:375], Ws["W2e1"],
                                     s2[32:64, n, 0:375],
                                     start=True, stop=False)
                    nc.tensor.matmul(psE[0:128, i, 0:375], Ws["W2e2"][:],
                                     s2[0:128, n, 1:376],
                                     start=False, stop=True)
                for i in range(2):
                    n = nb + i
                    nc.tensor.matmul(psO[0:128, i, 0:375], Ws["W2o1"][:],
                                     s2[0:128, n, 1:376],
                                     start=True, stop=False)
                    nc.tensor.matmul(psO[0:128, i, 0:375], Ws["W2o2"][:],
                                     s2[0:32, n, 2:377],
                                     start=False, stop=True)
                tE = tep.tile([128, 2, 384], BF16, tag="tE")
                nc.scalar.activation(tE[0:128, :, 0:375],
                                     psE[0:128, :, 0:375], ACTF.Relu,
                                     bias=B2t[:])
                nc.vector.scalar_tensor_tensor(
                    s3[0:128, nb:nb + 2, 1:376],
                    psO[0:128, :, 0:375], B2t[:], tE[0:128, :, 0:375],
                    AOP.add, AOP.max)

            def conv3(qd):
                # dense-P5: 5 accumulated matmuls, M=128 (firsts | seconds),
                # pooled pairs merge across the 0:64/64:128 partition halves
                blk, nq = qd // 2, (qd % 2) * 4
                n0 = blk * BN + nq
                ps = psp.tile([128, 2, 512], F32, tag="ps")
                pv = ps[0:128, 0, 0:500]
                nc.tensor.matmul(pv, Ws["W3P"][64:128, 0:128],
                                 s3[64:128, nq:nq + 4, 0:375:3],
                                 start=True, stop=False)
                for t in range(3):
                    c0 = 128 * (t + 1)
                    nc.tensor.matmul(pv, Ws["W3P"][:, c0:c0 + 128],
                                     s3[0:128, nq:nq + 4,
                                        t + 1:t + 376:3],
                                     start=False, stop=False)
                nc.tensor.matmul(pv, Ws["W3P"][0:64, 512:640],
                                 s3[0:64, nq:nq + 4, 4:379:3],
                                 start=False, stop=True)
                tE = tep.tile([128, 512], BF16, tag="tE3")
                nc.scalar.activation(tE[0:64, 0:500], ps[0:64, 0, 0:500],
                                     ACTF.Relu, bias=B3t[0:64])
                sec = ps[64:128, 0, 0:500].rearrange("p (n l) -> p n l", n=4)
                tEv = tE[0:64, 0:500].rearrange("p (n l) -> p n l", n=4)
                nc.vector.scalar_tensor_tensor(
                    s4[0:64, n0:n0 + 4, 0:63],
                    sec[:, :, 0:125:2], B3t[0:64],
                    tEv[:, :, 0:125:2], AOP.add, AOP.max)
                nc.vector.scalar_tensor_tensor(
                    s4[64:128, n0:n0 + 4, 0:62],
                    sec[:, :, 1:125:2], B3t[0:64],
                    tEv[:, :, 1:125:2], AOP.add, AOP.max)

            def fc(half):
                c0 = half * 64
                psfc = psp.tile([128, 2, 512], F32, tag="ps")
                for w in range(63):
                    nc.tensor.matmul(
                        psfc[0:100, 0, 0:64],
                        Ws["WFC"][:, w * 100:(w + 1) * 100],
                        s4[:, c0:c0 + 64, w], start=(w == 0), stop=(w == 62))
                nc.scalar.copy(p0sb[:, c0:c0 + 64], psfc[0:100, 0, 0:64])

            # pipeline: conv0(p) | conv1(p-1) | conv2(p-2) |
            #           conv3((p-3)/2 @ odd p) | fc halves at p=35/66
            # conv0 one step ahead of the nominal stagger: halos get two
            # full steps before conv1 consumes them
            for p in range(68):
                if not FLAGS["conv0"]:
                    pass
                elif p == 0:
                    conv0(0)
                    conv0(1)
                elif p + 1 < NPAIR:
                    conv0(p + 1)
                if p == 1:
                    issue_xw(2)
                if p >= 3 and (p + 1) % 4 == 0:
                    issue_xw((p + 1) // 4 + 2)
                if FLAGS["conv1"] and 0 <= p - 1 < NPAIR:
                    conv1(p - 1)
                if FLAGS["conv2"] and 0 <= p - 2 < NPAIR:
                    conv2(p - 2)
                if FLAGS["conv3"] and p % 2 == 0 and 0 <= (p - 4) // 2 < 32:
                    conv3((p - 4) // 2)
                if FLAGS["fc"] and (p == 35 or p == 67):
                    fc(0 if p == 35 else 1)
                for fn in dma_sched.get(p, ()):
                    fn()

            nc.sync.dma_start(out_p0[:], p0sb[:])

    nc.compile()
    return nc


# ------------------------------------------------------------- launch 2 ----

def _build_launch2():
    nc = bacc.Bacc("TRN2", target_bir_lowering=False, debug=False,
                   num_devices=NCORES)
    # packed inputs; matmul operands in bf16 (fp32 matmuls cost 4x)
    dA = nc.dram_tensor("PKA", [3, 916], BF16, kind="ExternalInput").ap()
    dB = nc.dram_tensor("PKB", [20, 701], BF16, kind="ExternalInput").ap()
    dC = nc.dram_tensor("PKC", [100, 131], F32, kind="ExternalInput").ap()
    dW = nc.dram_tensor("PKW", [100, 3], BF16, kind="ExternalInput").ap()
    out2 = nc.dram_tensor("out2", [2, NS], F32, kind="ExternalOutput").ap()

    with tile.TileContext(nc) as tc:
        with tc.tile_pool(name="w2p", bufs=1) as wp, \
             tc.tile_pool(name="ps2", bufs=2, space="PSUM") as psp:
            A = wp.tile([3, 916], BF16, tag="A")
            nc.sync.dma_start(A[:], dA[:])
            B = wp.tile([20, 701], BF16, tag="B")
            nc.sync.dma_start(B[:], dB[:])
            C = wp.tile([100, 131], F32, tag="C")
            nc.scalar.dma_start(C[:], dC[:])
            W = wp.tile([100, 3], BF16, tag="W")
            nc.scalar.dma_start(W[:], dW[:])

            # h1 = relu(wc @ feats + bc): [20, (t, n)]; relu on DVE (the
            # Act engine would pay a 1.3us activation-table load)
            h1 = wp.tile([20, 7 * NS], BF16, tag="h1")
            for half in range(2):
                c0 = half * 448
                ps = psp.tile([32, 448], F32, tag="ph")
                nc.tensor.matmul(ps[0:20, :], A[:, 896:916],
                                 A[:, c0:c0 + 448], start=True, stop=True)
                nc.vector.tensor_scalar(h1[:, c0:c0 + 448], ps[0:20, :],
                                        C[0:20, 129:130], 0.0,
                                        AOP.add, AOP.max)
            # z = relu(p0 + sum_t w0b_t.T @ h1_t + bl0)
            psz = psp.tile([100, NS], F32, tag="pz")
            for t in range(7):
                nc.tensor.matmul(psz[:], B[:, t * 100:(t + 1) * 100],
                                 h1[:, t * NS:(t + 1) * NS],
                                 start=(t == 0), stop=(t == 6))
            z = wp.tile([100, NS], BF16, tag="z")
            nc.vector.scalar_tensor_tensor(z[:], psz[:], C[:, 128:129],
                                           C[:, 0:128], AOP.add, AOP.add)
            nc.vector.tensor_scalar_max(z[:], z[:], 0.0)
            pso = psp.tile([32, NS], F32, tag="po")
            nc.tensor.matmul(pso[0:2, :], W[:, 0:2], z[:],
                             start=True, stop=True)
            osb = wp.tile([2, NS], F32, tag="osb")
            nc.vector.tensor_scalar(osb[:], pso[0:2, :], C[0:2, 130:131],
                                    None, AOP.add)
            nc.sync.dma_start(out2[:], osb[:])

    nc.compile()
    return nc


# --------------------------------------------------------------- kernel ----

def kernel(**inputs):
    ins = {k: np.asarray(v) for k, v in inputs.items()}
    x = ins["x"].astype(np.float32)

    if "l1" not in _CACHE:
        _CACHE["l1"] = _build_launch1()
    if "l2" not in _CACHE:
        _CACHE["l2"] = _build_launch2()
    w = _prep_weights(ins)

    xbf = x.astype(BF)
    # shared parity window: row (c, jj) = x[c, 8q - 1 + jj], jj in 0..9;
    # E reads rows jj=0..8, O reads jj=1..9, via two weight matrices
    xw = np.zeros((31, x.shape[0], 750), BF)
    for c in range(3):
        xw[c * 10 + 0, :, 1:750] = xbf[:, c, 7:5992:8]
        for jj in range(1, 9):
            xw[c * 10 + jj] = xbf[:, c, jj - 1::8]
        xw[c * 10 + 9, :, 0:749] = xbf[:, c, 8:6000:8]
    xw[30] = 1.0
    shards = [x[i * NS:(i + 1) * NS] for i in range(NCORES)]
    in1 = []
    for i, sh in enumerate(shards):
        sl = slice(i * NS, (i + 1) * NS)
        m = {"x_win": np.ascontiguousarray(xw[:, sl])}
        for nm in ["W0E", "W0O", "W1e1", "W1e2", "W1o1", "W1o2",
                   "W2e1", "W2e2", "W2o1", "W2o2", "W3P",
                   "WFC", "B2", "B3"]:
            m[nm] = w[nm]
        m["INIT1"] = np.ones((1, BN, 750), BF)
        m["INITZ"] = np.zeros((128, BN, 1), BF)
        m["INITZ4"] = np.zeros((128, BN, 4), BF)
        m["INITS4"] = np.zeros((64, NS, 1), BF)
        in1.append(m)
    t0 = time.time()
    res1 = run_bass_kernel_spmd(_CACHE["l1"], in1, list(range(NCORES)))
    LAST_EXEC_NS[0] = res1.exec_time_ns
    LAST_WALL_S[0] = time.time() - t0

    partial0 = np.concatenate(
        [res1.results[i]["partial0"].T for i in range(NCORES)], 0)

    # host: fp32 covariance (mirrors the reference einsum) + LAPACK-clone
    # eigh + global normalizers
    diff = x - x.mean(-1, keepdims=True, dtype=np.float32)
    cov = np.einsum("ncl,ndl->ncd", diff, diff,
                    dtype=np.float32).astype(np.float32)
    cov /= np.float32(L0 - 1)
    vals, vecs = _eigh3_batch(cov)
    covn = cov / np.abs(cov).max()
    valsn = (vals / vals.max())[..., None]
    feats = np.concatenate([covn, valsn, vecs], axis=-1).astype(np.float32)

    in2 = []
    for i in range(NCORES):
        sl = slice(i * NS, (i + 1) * NS)
        fT = feats[sl].transpose(1, 2, 0).reshape(3, 7 * NS)
        pkA = np.concatenate([fT, w["wcT"]], 1).astype(BF)
        pkB = np.concatenate(
            [np.concatenate([w["w0bT"][t] for t in range(7)], 1),
             w["bc"]], 1).astype(BF)
        bccol = np.zeros((100, 1), np.float32)
        bccol[0:20] = w["bc"]
        bl1col = np.zeros((100, 1), np.float32)
        bl1col[0:2] = w["bl1"]
        pkC = np.concatenate([partial0[sl].T, w["bl0"], bccol, bl1col],
                             1).astype(np.float32)
        pkW = np.zeros((100, 3), np.float32)
        pkW[:, 0:2] = w["wl1T"]
        pkW[0:2, 2:3] = w["bl1"]
        in2.append({"PKA": np.ascontiguousarray(pkA),
                    "PKB": np.ascontiguousarray(pkB),
                    "PKC": np.ascontiguousarray(pkC),
                    "PKW": pkW.astype(BF)})
    t0 = time.time()
    res2 = run_bass_kernel_spmd(_CACHE["l2"], in2, list(range(NCORES)))
    LAST_EXEC_NS[1] = res2.exec_time_ns
    LAST_WALL_S[1] = time.time() - t0

    out = np.concatenate([res2.results[i]["out2"].T for i in range(NCORES)],
                         0).astype(np.float32)
    return (out[:, 0:1], out[:, 1:2])



# revision 33
# speedup vs baseline: 1.0055x; 1.0055x over previous
"""Trainium2 Bass kernel for nn_BAZ_Network (dense CNN + cov/eig head).

Data-parallel over 8 NeuronCores: 128 samples each.

Launch 1 (per core), software-pipelined over 64 sample-pairs:
  conv trunk as G-packed banded-weight matmuls (bf16, fp32 PSUM), with
  conv biases folded into the matmuls via a ones-row in the rhs (conv0,
  conv1).  Postprocess per (E,O) parity pair is two fused ops over a
  2-sample two-PSUM-bank 3D access pattern:
    op1 (Act):  tE = relu(psE + b)           PSUM -> SBUF bf16
    op2 (DVE):  s  = max(psO + b, tE)        = relu(max(E,O)+b), the
                 maxpool, relu, bias and bf16 cast in one instruction.
  conv3 is dense-P5: one psum accumulates 5 chunked matmuls over the
  s3 pair-column layout (64/128/128/128/64 contraction rows); M=128
  packs pool-pair firsts in rows 0:60 and seconds in 64:124 so the
  maxpool merges across the aligned partition halves (junk rows are
  zero-weighted and killed by zero rows in WFC).  The FC contraction
  of the conv features against wl0[:, :7500] runs per-block.
  Stage stagger: conv0(p) | conv1(p-1) | conv2(p-2) | conv3 at even p |
  FC at p=35/67, which hides the halo-DMA and PSUM-evacuation latency.
Host: fp32 covariance (same einsum as the reference; cheaper than
  streaming x to the device a second time) + branch-exact fp32
  netlib-LAPACK ssyevd clone for the 3x3 eigh (required to reproduce
  jnp.linalg.eigh eigenvector signs).
Launch 2 (per core): eig-feature head: 1x1 conv (wc) + relu, remaining
  FC columns wl0[:, 7500:], bias+relu, final linear wl1.
"""

import os
import sys
import time
import numpy as np
import ml_dtypes

sys.path.insert(0, "/opt/trn_rl_repo")
os.environ["BASS_NEVER_TRACE"] = "1"

import concourse.bass as bass  # noqa: E402
import concourse.tile as tile  # noqa: E402
import concourse.mybir as mybir  # noqa: E402
from concourse import bacc  # noqa: E402
from concourse.bass_utils import run_bass_kernel_spmd  # noqa: E402

F32 = mybir.dt.float32
BF16 = mybir.dt.bfloat16
AOP = mybir.AluOpType
ACTF = mybir.ActivationFunctionType
BF = ml_dtypes.bfloat16

NCORES = 8
NS = 128          # samples per core
BN = 8            # samples per block
NBLK = NS // BN
NPAIR = NS // 2   # 64 sample-pairs, the pipeline unit
L0 = 6000

FLAGS = {"cov": True, "halo": True, "conv3": True, "fc": True,
         "conv0": True, "conv1": True, "conv2": True}
LAST_EXEC_NS = [None, None]
LAST_WALL_S = [None, None]
_CACHE = {}


# ---------------------------------------------------------------- eigh ----
# fp32 netlib-LAPACK ssyevd clone for n=3 (jobz='V', uplo='L').
# Matches jaxlib's CPU eigh (LAPACK >= 3.10 slartg) bit-closely: 0/3072
# eigenvector sign mismatches on the problem distribution.

_F = np.float32
_EPS = _F(np.finfo(np.float32).eps) * _F(0.5)
_EPS2 = _EPS * _EPS
_SAFMIN = _F(np.finfo(np.float32).tiny)


def _slapy2(x, y):
    xa, ya = abs(x), abs(y)
    w, z = max(xa, ya), min(xa, ya)
    if z == 0:
        return w
    return _F(w * _F(np.sqrt(_F(_F(1.0) + _F(_F(z / w) * _F(z / w))))))


def _sign(a, b):
    return abs(a) if b >= 0 else -abs(a)


def _slartg(f, g):
    if g == _F(0.0):
        return _F(1.0), _F(0.0), f
    if f == _F(0.0):
        return _F(0.0), _sign(_F(1.0), g), abs(g)
    d = _F(np.sqrt(_F(f * f + g * g)))
    c = _F(abs(f) / d)
    r = _sign(d, f)
    s = _F(g / r)
    return c, s, r


def _slaev2(a, b, c):
    sm = _F(a + c)
    df = _F(a - c)
    adf = abs(df)
    tb = _F(b + b)
    ab = abs(tb)
    acmx, acmn = (a, c) if abs(a) > abs(c) else (c, a)
    if adf > ab:
        t = _F(ab / adf)
        rt = _F(adf * _F(np.sqrt(_F(_F(1.0) + _F(t * t)))))
    elif adf < ab:
        t = _F(adf / ab)
        rt = _F(ab * _F(np.sqrt(_F(_F(1.0) + _F(t * t)))))
    else:
        rt = _F(ab * _F(np.sqrt(_F(2.0))))
    if sm < 0:
        rt1 = _F(_F(0.5) * _F(sm - rt))
        sgn1 = -1
        rt2 = _F(_F(_F(acmx / rt1) * acmn) - _F(_F(b / rt1) * b))
    elif sm > 0:
        rt1 = _F(_F(0.5) * _F(sm + rt))
        sgn1 = 1
        rt2 = _F(_F(_F(acmx / rt1) * acmn) - _F(_F(b / rt1) * b))
    else:
        rt1 = _F(_F(0.5) * rt)
        rt2 = _F(_F(-0.5) * rt)
        sgn1 = 1
    if df >= 0:
        cs = _F(df + rt)
        sgn2 = 1
    else:
        cs = _F(df - rt)
        sgn2 = -1
    acs = abs(cs)
    if acs > ab:
        ct = _F(-tb / cs)
        sn1 = _F(_F(1.0) / _F(np.sqrt(_F(_F(1.0) + _F(ct * ct)))))
        cs1 = _F(ct * sn1)
    else:
        if ab == 0:
            cs1, sn1 = _F(1.0), _F(0.0)
        else:
            tn = _F(-cs / tb)
            cs1 = _F(_F(1.0) / _F(np.sqrt(_F(_F(1.0) + _F(tn * tn)))))
            sn1 = _F(tn * cs1)
    if sgn1 == sgn2:
        cs1, sn1 = -sn1, cs1
    return rt1, rt2, cs1, sn1


def _ssytrd3(A):
    a00, a10, a20 = A[0, 0], A[1, 0], A[2, 0]
    a11, a21, a22 = A[1, 1], A[2, 1], A[2, 2]
    xnorm = abs(a20)
    if xnorm == _F(0.0):
        beta, v2, tau = a10, a20, _F(0.0)
    else:
        beta = -_sign(_slapy2(a10, xnorm), a10)
        tau = _F(_F(beta - a10) / beta)
        v2 = _F(a20 * _F(_F(1.0) / _F(a10 - beta)))
    e0 = beta
    if tau != _F(0.0):
        x0 = _F(_F(tau * a11) + _F(tau * _F(a21 * v2)))
        x1 = _F(_F(tau * a21) + _F(_F(tau * v2) * a22))
        sdot = _F(_F(x0 * _F(1.0)) + _F(x1 * v2))
        alpha = _F(_F(_F(-0.5) * tau) * sdot)
        w0 = _F(x0 + _F(alpha * _F(1.0)))
        w1 = _F(x1 + _F(alpha * v2))
        t1, t2 = -w0, _F(-1.0)
        a11 = _F(_F(a11 + _F(_F(1.0) * t1)) + _F(w0 * t2))
        a21 = _F(_F(a21 + _F(v2 * t1)) + _F(w1 * t2))
        t1b, t2b = -w1, -v2
        a22 = _F(_F(a22 + _F(v2 * t1b)) + _F(w1 * t2b))
    d = np.array([a00, a11, a22], np.float32)
    e = np.array([e0, a21, 0.0], np.float32)
    return d, e, v2, tau


def _ssteqr3(d, e):
    n = 3
    Z = np.eye(3, dtype=np.float32)
    wc = np.zeros(2, np.float32)
    ws = np.zeros(2, np.float32)
    nmaxit, jtot = 90, 0

    def lasr_b(l, m):
        for j in range(m - 1, l - 1, -1):
            c, s = wc[j - 1], ws[j - 1]
            if c != _F(1.0) or s != _F(0.0):
                for i in range(3):
                    t = Z[i, j]
                    Z[i, j] = _F(_F(c * t) - _F(s * Z[i, j - 1]))
                    Z[i, j - 1] = _F(_F(s * t) + _F(c * Z[i, j - 1]))

    def lasr_f(m, l):
        for j in range(m, l):
            c, s = wc[j - 1], ws[j - 1]
            if c != _F(1.0) or s != _F(0.0):
                for i in range(3):
                    t = Z[i, j]
                    Z[i, j] = _F(_F(c * t) - _F(s * Z[i, j - 1]))
                    Z[i, j - 1] = _F(_F(s * t) + _F(c * Z[i, j - 1]))

    l1 = 1
    while True:
        if l1 > n:
            break
        if l1 > 1:
            e[l1 - 2] = _F(0.0)
        m = n
        for mm in range(l1, n):
            tst = abs(e[mm - 1])
            if tst == _F(0.0):
                m = mm
                break
            if tst <= _F(_F(_F(np.sqrt(abs(d[mm - 1]))) *
                            _F(np.sqrt(abs(d[mm])))) * _EPS):
                e[mm - 1] = _F(0.0)
                m = mm
                break
        l = l1
        lend = m
        l1 = m + 1
        if lend == l:
            continue
        if abs(d[lend - 1]) < abs(d[l - 1]):
            lend, l = l, lend
        if lend > l:
            while True:  # QL
                m = lend
                if l != lend:
                    for mm in range(l, lend):
                        tst = _F(abs(e[mm - 1]) * abs(e[mm - 1]))
                        if tst <= _F(_F(_F(_EPS2 * abs(d[mm - 1])) *
                                        abs(d[mm])) + _SAFMIN):
                            m = mm
                            break
                if m < lend:
                    e[m - 1] = _F(0.0)
                p = d[l - 1]
                if m == l:
                    d[l - 1] = p
                    l += 1
                    if l <= lend:
                        continue
                    break
                if m == l + 1:
                    rt1, rt2, c, s = _slaev2(d[l - 1], e[l - 1], d[l])
                    wc[l - 1] = c
                    ws[l - 1] = s
                    lasr_b(l, l + 1)
                    d[l - 1] = rt1
                    d[l] = rt2
                    e[l - 1] = _F(0.0)
                    l += 2
                    if l <= lend:
                        continue
                    break
                if jtot == nmaxit:
                    break
                jtot += 1
                g = _F(_F(d[l] - p) / _F(_F(2.0) * e[l - 1]))
                r = _slapy2(g, _F(1.0))
                g = _F(_F(d[m - 1] - p) + _F(e[l - 1] / _F(g + _sign(r, g))))
                s = _F(1.0)
                c = _F(1.0)
                p = _F(0.0)
                for i in range(m - 1, l - 1, -1):
                    f = _F(s * e[i - 1])
                    b = _F(c * e[i - 1])
                    c, s, r = _slartg(g, f)
                    if i != m - 1:
                        e[i] = r
                    g = _F(d[i] - p)
                    r = _F(_F(_F(d[i - 1] - g) * s) + _F(_F(_F(2.0) * c) * b))
                    p = _F(s * r)
                    d[i] = _F(g + p)
                    g = _F(_F(c * r) - b)
                    wc[i - 1] = c
                    ws[i - 1] = -s
                lasr_b(l, m)
                d[l - 1] = _F(d[l - 1] - p)
                e[l - 1] = g
        else:
            while True:  # QR
                m = lend
                if l != lend:
                    for mm in range(l, lend, -1):
                        tst = _F(abs(e[mm - 2]) * abs(e[mm - 2]))
                        if tst <= _F(_F(_F(_EPS2 * abs(d[mm - 1])) *
                                        abs(d[mm - 2])) + _SAFMIN):
                            m = mm
                            break
                if m > lend:
                    e[m - 2] = _F(0.0)
                p = d[l - 1]
                if m == l:
                    d[l - 1] = p
                    l -= 1
                    if l >= lend:
                        continue
                    break
                if m == l - 1:
                    rt1, rt2, c, s = _slaev2(d[l - 2], e[l - 2], d[l - 1])
                    wc[m - 1] = c
                    ws[m - 1] = s
                    lasr_f(m, l)
                    d[l - 2] = rt1
                    d[l - 1] = rt2
                    e[l - 2] = _F(0.0)
                    l -= 2
                    if l >= lend:
                        continue
                    break
                if jtot == nmaxit:
                    break
                jtot += 1
                g = _F(_F(d[l - 2] - p) / _F(_F(2.0) * e[l - 2]))
                r = _slapy2(g, _F(1.0))
                g = _F(_F(d[m - 1] - p) + _F(e[l - 2] / _F(g + _sign(r, g))))
                s = _F(1.0)
                c = _F(1.0)
                p = _F(0.0)
                for i in range(m, l):
                    f = _F(s * e[i - 1])
                    b = _F(c * e[i - 1])
                    c, s, r = _slartg(g, f)
                    if i != m:
                        e[i - 2] = r
                    g = _F(d[i - 1] - p)
                    r = _F(_F(_F(d[i] - g) * s) + _F(_F(_F(2.0) * c) * b))
                    p = _F(s * r)
                    d[i - 1] = _F(g + p)
                    g = _F(_F(c * r) - b)
                    wc[i - 1] = c
                    ws[i - 1] = s
                lasr_f(m, l)
                d[l - 1] = _F(d[l - 1] - p)
                e[l - 2] = g
        if jtot >= nmaxit:
            break
    for ii in range(2, n + 1):
        i = ii - 1
        k = i
        p = d[i - 1]
        for j in range(ii, n + 1):
            if d[j - 1] < p:
                k = j
                p = d[j - 1]
        if k != i:
            d[k - 1] = d[i - 1]
            d[i - 1] = p
            tmp = Z[:, k - 1].copy()
            Z[:, k - 1] = Z[:, i - 1]
            Z[:, i - 1] = tmp
    return d, Z


def _eigh3_batch(covs):
    n = covs.shape[0]
    W = np.empty((n, 3), np.float32)
    V = np.empty((n, 3, 3), np.float32)
    for i in range(n):
        d, e, v2, tau = _ssytrd3(covs[i])
        w, Z = _ssteqr3(d, e)
        if tau != _F(0.0):
            for j in range(3):
                vtz = _F(Z[1, j] + _F(v2 * Z[2, j]))
                tvz = _F(tau * vtz)
                Z[1, j] = _F(Z[1, j] - tvz)
                Z[2, j] = _F(Z[2, j] - _F(v2 * tvz))
        W[i] = w
        V[i] = Z
    return W, V


# ------------------------------------------------------------- weights ----

def _prep_weights(ins):
    """Host-side packing of the model weights into device layouts.

    Strided column-pair scheme (see the layer maps below); conv0/conv1
    biases ride a ones-row in the rhs so the matmul itself adds them.
    """
    w0, w1, w2, w3 = ins["w0"], ins["w1"], ins["w2"], ins["w3"]
    b0, b1 = np.asarray(ins["b0"], np.float32), np.asarray(ins["b1"],
                                                           np.float32)

    d = {}
    # conv0: window rows (c:3, j:9): even cols l = 8q-1+j, odd l = 8q+j;
    # k = j - 2g for output slot g; row 27 = ones -> bias.
    W0E = np.zeros((31, 80), np.float32)
    W0O = np.zeros((31, 80), np.float32)
    for c in range(3):
        for jj in range(10):
            for g in range(4):
                for o in range(20):
                    k = jj - 2 * g
                    if 0 <= k < 3:
                        W0E[c * 10 + jj, g * 20 + o] = w0[o, c, k]
                    k = jj - 1 - 2 * g
                    if 0 <= k < 3:
                        W0O[c * 10 + jj, g * 20 + o] = w0[o, c, k]
    W0E[30, :] = np.tile(b0, 4)
    W0O[30, :] = np.tile(b0, 4)
    d["W0E"] = W0E.astype(BF)
    d["W0O"] = W0O.astype(BF)

    def s1_rows(with_hl, with_hr):
        rows = [(g * 20, 20, g) for g in range(4)]
        if with_hl:
            rows.append((80, 20, -1))
        if with_hr:
            rows.append((100, 20, 4))
        return rows

    def mk(w, blocks, Ghalf, parity, Cout, shift, colbase=None):
        Cin = w.shape[1]
        K = max(rb + Cin for rb, _, _ in blocks)
        if colbase is None:
            colbase = [g * Cout for g in range(Ghalf)]
        W = np.zeros((K, max(colbase) + Cout), np.float32)
        for rb, _, lrel in blocks:
            for g in range(Ghalf):
                pos = 2 * g + parity
                k = (lrel + shift) - pos + 1
                if 0 <= k < 3:
                    for ci in range(Cin):
                        W[rb + ci, colbase[g] + np.arange(Cout)] = w[:, ci, k]
        return W

    # conv1 output M-order: g0->0, g1->64, g2->96, g3->32 (C1B) so conv2's
    # boundary reads sit at legal rhs bases.
    C1B = [0, 64, 96, 32]

    # baseline-layout W1 blocks, then re-rowed for the s1 layout with the
    # ones row at 80: main [0:80], ones 80, hl [81:101], hr [101:121].
    w1e1_base = mk(w1, s1_rows(True, False), 4, 0, 32, 0, C1B)   # [100,128]
    w1e2 = mk(w1, [(rb, 20, lr + 4) for rb, _, lr in
                   s1_rows(False, False)], 4, 0, 32, 0, C1B)     # [80,128]
    w1o1 = mk(w1, s1_rows(False, False), 4, 1, 32, 0, C1B)       # [80,128]
    w1o2_blocks = ([(g * 20, 20, g + 4) for g in range(4)] +
                   [(80, 20, 1000), (100, 20, 8)])
    w1o2_base = mk(w1, w1o2_blocks, 4, 1, 32, 0, C1B)            # [120,128]
    b1t = np.tile(b1, 4)
    W1e1 = np.zeros((101, 128), np.float32)
    W1e1[0:80] = w1e1_base[0:80]
    W1e1[80] = b1t
    W1e1[81:101] = w1e1_base[80:100]
    W1o2 = np.zeros((121, 128), np.float32)
    W1o2[0:80] = w1o2_base[0:80]
    W1o2[80] = b1t
    W1o2[101:121] = w1o2_base[100:120]
    d["W1e1"] = W1e1.astype(BF)
    d["W1e2"] = w1e2.astype(BF)
    d["W1o1"] = w1o1.astype(BF)
    d["W1o2"] = W1o2.astype(BF)

    # conv2 (G=4, Ghalf=2, Cout=64): stored2 rows (g:4, o:32)->128
    s2_main = [(0, 32, 0), (64, 32, 1), (96, 32, 2), (32, 32, 3)]
    d["W2e1"] = mk(w2, [(0, 32, -1)], 2, 0, 64, 0).astype(BF)
    d["W2e2"] = mk(w2, s2_main, 2, 0, 64, 0).astype(BF)
    d["W2o1"] = mk(w2, s2_main, 2, 1, 64, 0).astype(BF)
    d["W2o2"] = mk(w2, [(0, 32, 4)], 2, 1, 64, 0).astype(BF)

    # conv3 dense-P5: out block u = dense positions {6u..6u+5}, window
    # {6u-1..6u+6} read as 5 accumulated chunks from s3 pair-columns
    # (col v = positions {2v-2, 2v-1}; g=0 rows 0:64, g=1 rows 64:128):
    #   p0: col 3u   g1 (pos 6u-1),  p1: col 3u+1 (6u, 6u+1),
    #   p2: col 3u+2 (6u+2, 6u+3),   p3: col 3u+3 (6u+4, 6u+5),
    #   p4: col 3u+4 g0 (pos 6u+6).
    # M = 128: rows 0-59 firsts-of-pool-pairs (j = 2*j2), 64-123 seconds
    # (j = 2*j2+1), 60-63/124-127 zero-weight junk (killed by WFC zeros).
    w3p = np.zeros((128, 5 * 128), np.float32)
    PASS_POS = [{1: -1}, {0: 0, 1: 1}, {0: 2, 1: 3}, {0: 4, 1: 5}, {0: 6}]
    for pi, gmap in enumerate(PASS_POS):
        for g, pos_rel in gmap.items():
            for m in range(128):
                sub, up = m % 64, m // 64
                if sub >= 60:
                    continue
                j = 2 * (sub // 20) + up
                k = pos_rel - j + 1
                if 0 <= k < 3:
                    o = sub % 20
                    for ci in range(64):
                        w3p[g * 64 + ci, pi * 128 + m] = w3[o, ci, k]
    d["W3P"] = w3p.astype(BF)

    # fc: s4 rows (up:2 pad64, j2:3, o:20), col w: feature (o, l4=6w+3up+j2)
    wl0 = ins["wl0"]
    WFC = np.zeros((128, 63 * 100), np.float32)
    for w in range(63):
        for up in range(2):
            for j2 in range(3):
                l4 = 6 * w + 3 * up + j2
                if l4 < 375:
                    for o in range(20):
                        WFC[up * 64 + j2 * 20 + o, w * 100:(w + 1) * 100] = \
                            wl0[:, o * 375 + l4]
    d["WFC"] = WFC.astype(BF)

    d["B2"] = np.tile(ins["b2"], 2).astype(np.float32)[:, None]   # [128]
    b3h = np.concatenate([np.tile(ins["b3"], 3), np.zeros(4)])
    d["B3"] = np.concatenate([b3h, b3h]).astype(np.float32)[:, None]  # [128]
    # launch 2
    d["wcT"] = ins["wc"][:, :, 0].T.astype(np.float32).copy()      # [3, 20]
    d["bc"] = ins["bc"].astype(np.float32)[:, None]                # [20, 1]
    w0b = np.zeros((7, 20, 100), np.float32)
    for t in range(7):
        for o in range(20):
            w0b[t, o] = ins["wl0"][:, 7500 + o * 7 + t]
    d["w0bT"] = w0b
    d["bl0"] = ins["bl0"].astype(np.float32)[:, None]              # [100, 1]
    d["wl1T"] = ins["wl1"].T.astype(np.float32).copy()             # [100, 2]
    d["bl1"] = ins["bl1"].astype(np.float32)[:, None]              # [2, 1]
    return d


# ------------------------------------------------------------- launch 1 ----

def _build_launch1():
    nc = bacc.Bacc("TRN2", target_bir_lowering=False, debug=False,
                   num_devices=NCORES)
    dram = {}
    for nm, shape, dt in [
        ("x_win", [31, NS, 750], BF16),
        ("W0E", [31, 80], BF16), ("W0O", [31, 80], BF16),
        ("W1e1", [101, 128], BF16), ("W1e2", [80, 128], BF16),
        ("W1o1", [80, 128], BF16), ("W1o2", [121, 128], BF16),
        ("W2e1", [32, 128], BF16), ("W2e2", [128, 128], BF16),
        ("W2o1", [128, 128], BF16), ("W2o2", [32, 128], BF16),
        ("W3P", [128, 640], BF16),
        ("WFC", [128, 6300], BF16),
        ("B2", [128, 1], F32), ("B3", [128, 1], F32),
        ("INIT1", [1, BN, 750], BF16), ("INITZ", [128, BN, 1], BF16),
        ("INITZ4", [128, BN, 4], BF16), ("INITS4", [64, NS, 1], BF16),
    ]:
        dram[nm] = nc.dram_tensor(nm, shape, dt, kind="ExternalInput").ap()
    out_p0 = nc.dram_tensor("partial0", [100, NS], F32,
                            kind="ExternalOutput").ap()

    with tile.TileContext(nc) as tc:
        with tc.tile_pool(name="wpool", bufs=1) as wp, \
             tc.tile_pool(name="xw", bufs=2) as xwp, \
             tc.tile_pool(name="s1p", bufs=1) as s1p, \
             tc.tile_pool(name="s2p", bufs=1) as s2p, \
             tc.tile_pool(name="s3p", bufs=1) as s3p, \
             tc.tile_pool(name="s4p", bufs=1) as s4p, \
             tc.tile_pool(name="tep", bufs=4) as tep, \
             tc.tile_pool(name="ps", bufs=4, space="PSUM") as psp:

            xw_tiles = {}

            def issue_xw(b, split=False):
                if b >= NBLK or b in xw_tiles:
                    return
                n0 = b * BN
                tw = xwp.tile([31, BN, 750], BF16, tag="xw")
                if split:
                    # block 0: land the first pair's samples ahead of the
                    # rest so conv0(0) is not gated on the full 8-sample DMA
                    nc.sync.dma_start(tw[:, 0:2, :],
                                      dram["x_win"][:, n0:n0 + 2, :])
                    nc.sync.dma_start(tw[:, 2:BN, :],
                                      dram["x_win"][:, n0 + 2:n0 + BN, :])
                else:
                    nc.sync.dma_start(tw[:], dram["x_win"][:, n0:n0 + BN, :])
                xw_tiles[b] = tw

            # The SP DMA queue is FIFO and a DMA holds HWDGE ~0.6us each:
            # preload ONLY what conv0(0)/conv1(0) need, trickle the rest
            # into the pipeline steps below via dma_sched.
            p0sb = wp.tile([100, NS], F32, tag="p0sb")
            s1 = s1p.tile([121, BN, 750], BF16, tag="s1")
            s2 = s2p.tile([128, BN, 377], BF16, tag="s2")
            s3 = s3p.tile([128, BN, 380], BF16, tag="s3")
            s4 = s4p.tile([128, NS, 63], BF16, tag="s4")

            Ws = {}
            for nm in ["W1e1", "W1e2", "W1o1", "W1o2", "W0E", "W0O",
                       "W2e2", "W2o1", "W2o2", "W3P", "WFC"]:
                Ws[nm] = wp.tile(list(dram[nm].shape), BF16, name=nm, tag=nm)
            Wpad = wp.tile([64, 128], BF16, name="W2e1", tag="W2e1")
            Ws["W2e1"] = Wpad[32:64]
            B2t = wp.tile([128, 1], F32, tag="B2")
            B3t = wp.tile([128, 1], F32, tag="B3")

            def dma_w(nm):
                nc.sync.dma_start(Ws[nm][:], dram[nm][:])

            # prologue: conv0(0..1) + conv1(0) prerequisites only
            dma_w("W0E")
            dma_w("W0O")
            issue_xw(0, split=True)
            issue_xw(1)
            # ones row + hl/hr edges (engines cannot address partition
            # bases off the 0/32/64/96 grid; DMA can); conv1 prereqs ride
            # the otherwise-idle Act HWDGE queue in parallel
            nc.scalar.dma_start(s1[80:81, :, :], dram["INIT1"][:])
            nc.scalar.dma_start(s1[81:101, :, 0:1], dram["INITZ"][0:20])
            nc.scalar.dma_start(s1[101:121, :, 749:750], dram["INITZ"][0:20])
            for nm in ["W1e1", "W1e2", "W1o1", "W1o2"]:
                nc.scalar.dma_start(Ws[nm][:], dram[nm][:])

            def dmas_step0():
                nc.sync.dma_start(Wpad[32:64], dram["W2e1"][:])
                for nm in ["W2e2", "W2o1", "W2o2"]:
                    dma_w(nm)
                nc.sync.dma_start(B2t[:], dram["B2"][:])
                nc.sync.dma_start(s2[:, :, 0:1], dram["INITZ"][:])
                nc.sync.dma_start(s2[:, :, 376:377], dram["INITZ"][:])

            def dmas_step1():
                dma_w("W3P")
                nc.sync.dma_start(s3[:, :, 0:1], dram["INITZ"][:])

            def dmas_step2():
                nc.sync.dma_start(B3t[:], dram["B3"][:])
                nc.sync.dma_start(s3[:, :, 376:380], dram["INITZ4"][:])
                nc.sync.dma_start(s4[64:128, :, 62:63], dram["INITS4"][:])

            def make_wfc_piece(i):
                c0 = i * 1575
                return lambda: nc.scalar.dma_start(
                    Ws["WFC"][:, c0:c0 + 1575], dram["WFC"][:, c0:c0 + 1575])

            dma_sched = {}
            dma_sched.setdefault(0, []).append(dmas_step0)
            dma_sched.setdefault(1, []).append(dmas_step1)
            dma_sched.setdefault(2, []).append(dmas_step2)
            for i in range(4):
                dma_sched.setdefault(26 + 2 * i, []).append(make_wfc_piece(i))

            def conv0(p):
                blk, nb = p // 4, (p % 4) * 2
                xwt = xw_tiles[blk]
                for ch in range(2):
                    c0 = ch * 375
                    psE = psp.tile([128, 2, 512], F32, tag="ps")
                    psO = psp.tile([128, 2, 512], F32, tag="ps")
                    for i in range(2):
                        nc.tensor.matmul(psE[0:80, i, 0:375], Ws["W0E"][:],
                                         xwt[:, nb + i, c0:c0 + 375],
                                         start=True, stop=True)
                    for i in range(2):
                        nc.tensor.matmul(psO[0:80, i, 0:375], Ws["W0O"][:],
                                         xwt[:, nb + i, c0:c0 + 375],
                                         start=True, stop=True)
                    tE = tep.tile([128, 2, 384], BF16, tag="tE")
                    nc.scalar.activation(tE[0:80, :, 0:375],
                                         psE[0:80, :, 0:375], ACTF.Relu)
                    nc.vector.scalar_tensor_tensor(
                        s1[0:80, nb:nb + 2, c0:c0 + 375],
                        psO[0:80, :, 0:375], 0.0, tE[0:80, :, 0:375],
                        AOP.max, AOP.max)
                # per-pair halo rows for conv1
                if FLAGS["halo"]:
                    nc.sync.dma_start(s1[81:101, nb:nb + 2, 1:750],
                                      s1[60:80, nb:nb + 2, 0:749])
                    nc.sync.dma_start(s1[101:121, nb:nb + 2, 0:749],
                                      s1[0:20, nb:nb + 2, 1:750])

            def conv1(p):
                nb = (p % 4) * 2
                psE = psp.tile([128, 2, 512], F32, tag="ps")
                psO = psp.tile([128, 2, 512], F32, tag="ps")
                for i in range(2):
                    n = nb + i
                    nc.tensor.matmul(psE[0:128, i, 0:375], Ws["W1e1"][:],
                                     s1[0:101, n, 0:750:2],
                                     start=True, stop=False)
                    nc.tensor.matmul(psE[0:128, i, 0:375], Ws["W1e2"][:],
                                     s1[0:80, n, 1:750:2],
                                     start=False, stop=True)
                for i in range(2):
                    n = nb + i
                    nc.tensor.matmul(psO[0:128, i, 0:375], Ws["W1o1"][:],
                                     s1[0:80, n, 0:750:2],
                                     start=True, stop=False)
                    nc.tensor.matmul(psO[0:128, i, 0:375], Ws["W1o2"][:],
                                     s1[0:121, n, 1:750:2],
                                     start=False, stop=True)
                tE = tep.tile([128, 2, 384], BF16, tag="tE")
                nc.scalar.activation(tE[0:128, :, 0:375],
                                     psE[0:128, :, 0:375], ACTF.Relu)
                nc.vector.scalar_tensor_tensor(
                    s2[0:128, nb:nb + 2, 1:376],
                    psO[0:128, :, 0:375], 0.0, tE[0:128, :, 0:375],
                    AOP.max, AOP.max)

            def conv2(p):
                nb = (p % 4) * 2
                psE = psp.tile([128, 2, 512], F32, tag="ps")
                psO = psp.tile([128, 2, 512], F32, tag="ps")
                for i in range(2):
                    n = nb + i
                    nc.tensor.matmul(psE[0:128, i, 0:375], Ws["W2e1"],
                                     s2[32:64, n, 0:375],
                                     start=True, stop=False)
                    nc.tensor.matmul(psE[0:128, i, 0:375], Ws["W2e2"][:],
                                     s2[0:128, n, 1:376],
                                     start=False, stop=True)
                for i in range(2):
                    n = nb + i
                    nc.tensor.matmul(psO[0:128, i, 0:375], Ws["W2o1"][:],
                                     s2[0:128, n, 1:376],
                                     start=True, stop=False)
                    nc.tensor.matmul(psO[0:128, i, 0:375], Ws["W2o2"][:],
                                     s2[0:32, n, 2:377],
                                     start=False, stop=True)
                tE = tep.tile([128, 2, 384], BF16, tag="tE")
                nc.scalar.activation(tE[0:128, :, 0:375],
                                     psE[0:128, :, 0:375], ACTF.Relu,
                                     bias=B2t[:])
                nc.vector.scalar_tensor_tensor(
                    s3[0:128, nb:nb + 2, 1:376],
                    psO[0:128, :, 0:375], B2t[:], tE[0:128, :, 0:375],
                    AOP.add, AOP.max)

            def conv3(qd):
                # dense-P5: 5 accumulated matmuls, M=128 (firsts | seconds),
                # pooled pairs merge across the 0:64/64:128 partition halves
                blk, nq = qd // 2, (qd % 2) * 4
                n0 = blk * BN + nq
                ps = psp.tile([128, 2, 512], F32, tag="ps")
                pv = ps[0:128, 0, 0:500]
                nc.tensor.matmul(pv, Ws["W3P"][64:128, 0:128],
                                 s3[64:128, nq:nq + 4, 0:375:3],
                                 start=True, stop=False)
                for t in range(3):
                    c0 = 128 * (t + 1)
                    nc.tensor.matmul(pv, Ws["W3P"][:, c0:c0 + 128],
                                     s3[0:128, nq:nq + 4,
                                        t + 1:t + 376:3],
                                     start=False, stop=False)
                nc.tensor.matmul(pv, Ws["W3P"][0:64, 512:640],
                                 s3[0:64, nq:nq + 4, 4:379:3],
                                 start=False, stop=True)
                tE = tep.tile([128, 512], BF16, tag="tE3")
                nc.scalar.activation(tE[0:64, 0:500], ps[0:64, 0, 0:500],
                                     ACTF.Relu, bias=B3t[0:64])
                sec = ps[64:128, 0, 0:500].rearrange("p (n l) -> p n l", n=4)
                tEv = tE[0:64, 0:500].rearrange("p (n l) -> p n l", n=4)
                nc.vector.scalar_tensor_tensor(
                    s4[0:64, n0:n0 + 4, 0:63],
                    sec[:, :, 0:125:2], B3t[0:64],
                    tEv[:, :, 0:125:2], AOP.add, AOP.max)
                nc.vector.scalar_tensor_tensor(
                    s4[64:128, n0:n0 + 4, 0:62],
                    sec[:, :, 1:125:2], B3t[0:64],
                    tEv[:, :, 1:125:2], AOP.add, AOP.max)

            def fc(half):
                c0 = half * 64
                psfc = psp.tile([128, 2, 512], F32, tag="ps")
                for w in range(63):
                    nc.tensor.matmul(
                        psfc[0:100, 0, 0:64],
                        Ws["WFC"][:, w * 100:(w + 1) * 100],
                        s4[:, c0:c0 + 64, w], start=(w == 0), stop=(w == 62))
                nc.scalar.copy(p0sb[:, c0:c0 + 64], psfc[0:100, 0, 0:64])

            # pipeline: conv0(p) | conv1(p-1) | conv2(p-2) |
            #           conv3((p-3)/2 @ odd p) | fc halves at p=35/66
            # conv0 one step ahead of the nominal stagger: halos get two
            # full steps before conv1 consumes them
            for p in range(68):
                if not FLAGS["conv0"]:
                    pass
                elif p == 0:
                    conv0(0)
                    conv0(1)
                elif p + 1 < NPAIR:
                    conv0(p + 1)
                if p == 1:
                    issue_xw(2)
                if p >= 3 and (p + 1) % 4 == 0:
                    issue_xw((p + 1) // 4 + 2)
                if FLAGS["conv1"] and 0 <= p - 1 < NPAIR:
                    conv1(p - 1)
                if FLAGS["conv2"] and 0 <= p - 2 < NPAIR:
                    conv2(p - 2)
                if FLAGS["conv3"] and p % 2 == 0 and 0 <= (p - 4) // 2 < 32:
                    conv3((p - 4) // 2)
                if FLAGS["fc"] and (p == 35 or p == 67):
                    fc(0 if p == 35 else 1)
                for fn in dma_sched.get(p, ()):
                    fn()

            nc.sync.dma_start(out_p0[:], p0sb[:])

    nc.compile()
    return nc


# ------------------------------------------------------------- launch 2 ----

def _build_launch2():
    nc = bacc.Bacc("TRN2", target_bir_lowering=False, debug=False,
                   num_devices=NCORES)
    # packed inputs; matmul operands in bf16 (fp32 matmuls cost 4x)
    dA = nc.dram_tensor("PKA", [3, 916], BF16, kind="ExternalInput").ap()
    dB = nc.dram_tensor("PKB", [20, 701], BF16, kind="ExternalInput").ap()
    dC = nc.dram_tensor("PKC", [100, 131], F32, kind="ExternalInput").ap()
    dW = nc.dram_tensor("PKW", [100, 3], BF16, kind="ExternalInput").ap()
    out2 = nc.dram_tensor("out2", [2, NS], F32, kind="ExternalOutput").ap()

    with tile.TileContext(nc) as tc:
        with tc.tile_pool(name="w2p", bufs=1) as wp, \
             tc.tile_pool(name="ps2", bufs=2, space="PSUM") as psp:
            A = wp.tile([3, 916], BF16, tag="A")
            nc.sync.dma_start(A[:], dA[:])
            B = wp.tile([20, 701], BF16, tag="B")
            nc.sync.dma_start(B[:], dB[:])
            C = wp.tile([100, 131], F32, tag="C")
            nc.scalar.dma_start(C[:], dC[:])
            W = wp.tile([100, 3], BF16, tag="W")
            nc.scalar.dma_start(W[:], dW[:])

            # h1 = relu(wc @ feats + bc): [20, (t, n)]; relu on DVE (the
            # Act engine would pay a 1.3us activation-table load)
            h1 = wp.tile([20, 7 * NS], BF16, tag="h1")
            for half in range(2):
                c0 = half * 448
                ps = psp.tile([32, 448], F32, tag="ph")
                nc.tensor.matmul(ps[0:20, :], A[:, 896:916],
                                 A[:, c0:c0 + 448], start=True, stop=True)
                nc.vector.tensor_scalar(h1[:, c0:c0 + 448], ps[0:20, :],
                                        C[0:20, 129:130], 0.0,
                                        AOP.add, AOP.max)
            # z = relu(p0 + sum_t w0b_t.T @ h1_t + bl0)
            psz = psp.tile([100, NS], F32, tag="pz")
            for t in range(7):
                nc.tensor.matmul(psz[:], B[:, t * 100:(t + 1) * 100],
                                 h1[:, t * NS:(t + 1) * NS],
                                 start=(t == 0), stop=(t == 6))
            z = wp.tile([100, NS], BF16, tag="z")
            nc.vector.scalar_tensor_tensor(z[:], psz[:], C[:, 128:129],
                                           C[:, 0:128], AOP.add, AOP.add)
            nc.vector.tensor_scalar_max(z[:], z[:], 0.0)
            pso = psp.tile([32, NS], F32, tag="po")
            nc.tensor.matmul(pso[0:2, :], W[:, 0:2], z[:],
                             start=True, stop=True)
            osb = wp.tile([2, NS], F32, tag="osb")
            nc.vector.tensor_scalar(osb[:], pso[0:2, :], C[0:2, 130:131],
                                    None, AOP.add)
            nc.sync.dma_start(out2[:], osb[:])

    nc.compile()
    return nc


# --------------------------------------------------------------- kernel ----

def kernel(**inputs):
    ins = {k: np.asarray(v) for k, v in inputs.items()}
    x = ins["x"].astype(np.float32)

    if "l1" not in _CACHE:
        _CACHE["l1"] = _build_launch1()
    if "l2" not in _CACHE:
        _CACHE["l2"] = _build_launch2()
    w = _prep_weights(ins)

    xbf = x.astype(BF)
    # shared parity window: row (c, jj) = x[c, 8q - 1 + jj], jj in 0..9;
    # E reads rows jj=0..8, O reads jj=1..9, via two weight matrices
    xw = np.zeros((31, x.shape[0], 750), BF)
    for c in range(3):
        xw[c * 10 + 0, :, 1:750] = xbf[:, c, 7:5992:8]
        for jj in range(1, 9):
            xw[c * 10 + jj] = xbf[:, c, jj - 1::8]
        xw[c * 10 + 9, :, 0:749] = xbf[:, c, 8:6000:8]
    xw[30] = 1.0
    shards = [x[i * NS:(i + 1) * NS] for i in range(NCORES)]
    in1 = []
    for i, sh in enumerate(shards):
        sl = slice(i * NS, (i + 1) * NS)
        m = {"x_win": np.ascontiguousarray(xw[:, sl])}
        for nm in ["W0E", "W0O", "W1e1", "W1e2", "W1o1", "W1o2",
                   "W2e1", "W2e2", "W2o1", "W2o2", "W3P",
                   "WFC", "B2", "B3"]:
            m[nm] = w[nm]
        m["INIT1"] = np.ones((1, BN, 750), BF)
        m["INITZ"] = np.zeros((128, BN, 1), BF)
        m["INITZ4"] = np.zeros((128, BN, 4), BF)
        m["INITS4"] = np.zeros((64, NS, 1), BF)
        in1.append(m)
    t0 = time.time()
    res1 = run_bass_kernel_spmd(_CACHE["l1"], in1, list(range(NCORES)))
    LAST_EXEC_NS[0] = res1.exec_time_ns
    LAST_WALL_S[0] = time.time() - t0

    partial0 = np.concatenate(
        [res1.results[i]["partial0"].T for i in range(NCORES)], 0)

    # host: fp32 covariance (mirrors the reference einsum) + LAPACK-clone
    # eigh + global normalizers
    diff = x - x.mean(-1, keepdims=True, dtype=np.float32)
    cov = np.einsum("ncl,ndl->ncd", diff, diff,
                    dtype=np.float32).astype(np.float32)
    cov /= np.float32(L0 - 1)
    vals, vecs = _eigh3_batch(cov)
    covn = cov / np.abs(cov).max()
    valsn = (vals / vals.max())[..., None]
    feats = np.concatenate([covn, valsn, vecs], axis=-1).astype(np.float32)

    in2 = []
    for i in range(NCORES):
        sl = slice(i * NS, (i + 1) * NS)
        fT = feats[sl].transpose(1, 2, 0).reshape(3, 7 * NS)
        pkA = np.concatenate([fT, w["wcT"]], 1).astype(BF)
        pkB = np.concatenate(
            [np.concatenate([w["w0bT"][t] for t in range(7)], 1),
             w["bc"]], 1).astype(BF)
        bccol = np.zeros((100, 1), np.float32)
        bccol[0:20] = w["bc"]
        bl1col = np.zeros((100, 1), np.float32)
        bl1col[0:2] = w["bl1"]
        pkC = np.concatenate([partial0[sl].T, w["bl0"], bccol, bl1col],
                             1).astype(np.float32)
        pkW = np.zeros((100, 3), np.float32)
        pkW[:, 0:2] = w["wl1T"]
        pkW[0:2, 2:3] = w["bl1"]
        in2.append({"PKA": np.ascontiguousarray(pkA),
                    "PKB": np.ascontiguousarray(pkB),
                    "PKC": np.ascontiguousarray(pkC),
                    "PKW": pkW.astype(BF)})
    t0 = time.time()
    res2 = run_bass_kernel_spmd(_CACHE["l2"], in2, list(range(NCORES)))
    LAST_EXEC_NS[1] = res2.exec_time_ns
    LAST_WALL_S[1] = time.time() - t0

    out = np.concatenate([res2.results[i]["out2"].T for i in range(NCORES)],
                         0).astype(np.float32)
    return (out[:, 0:1], out[:, 1:2])

